# revision 82
# baseline (speedup 1.0000x reference)
"""Multi-head attention (B=4, S=2048, D=1024, H=16) on 8 TRN2 NeuronCores.

Sharding: core = (batch b = core//2, head-group g = core%2). Each core runs
8 heads (512 channels) of one batch element end-to-end; the host sums the two
head-group partials per batch and adds the constant bias term (bo + Wo@bv).

v2: all matmul operands bf16 (1 cyc/row on PE at any free size, half DMA);
causal offsets exact at 128 granularity; softmax reciprocal via ACT
exp(-ln(den)) (no DVE 8-cyc/elem reciprocal); projections of K/V token-blocks
and the output projection are emitted as "quanta" interleaved into the
attention loop so the PE never idles (and never HAM-rethrottles) while the
ACT engine works through the exp stream.

Device layouts (host-prepped):
  xqt/xkt/xvt [1024, 2048] bf16   input.T per batch
  wqt/wkt/wvt [128, 8, 512] bf16  W_slice.T as [d_par, d_chunk, c]
  wot         [128, 4, 1024] bf16 WoT_slice as [c_par, c_chunk, dout]
  bq/bk       [128, 4] f32        per-partition bias (pre-softmax biases only)
  tria        [128, 128] bf16     16*(i<=k)      -- causal ramp, lhsT
  trib        [128, 4, 512] bf16  -15*(i+128j>q) -- causal ramp, rhs
Output: out_p [2048, 1024] bf16 partial (no bias).
"""

from collections import deque
from contextlib import ExitStack

import numpy as np

import concourse.bacc as bacc
import concourse.bass as bass
import concourse.mybir as mybir
import concourse.tile as tile
from concourse.bass_utils import run_bass_kernel_spmd

B, S, D, H, DK = 4, 2048, 1024, 16, 64
HL, C = 8, 512  # heads / channels per core
NCORES = 8
TB = 512  # token block for projections
QB = 512  # query block for attention
NTB = S // TB  # 4
NKC = S // 128  # 16 key chunks
DCH = D // 128  # 8 d-chunks
F32 = mybir.dt.float32
BF16 = mybir.dt.bfloat16
AF = mybir.ActivationFunctionType


def _patch_act_tables():
    """Confine Exp/Identity/Ln to the natural_log_exp_and_others table set so
    the act-table-load pass emits a single ACT_TABLE_LOAD instead of
    ping-ponging between the exp set and the ln set on every softmax
    normalization (measured 33 loads = 42us of ScalarE time). Only set
    *contents* are edited -- dict order (the act_func_set_id space) is kept."""
    import concourse.bacc as bacc_mod
    import concourse.hw_specs as hw_specs

    if getattr(bacc_mod, "_act_tables_patched", False):
        return
    orig = hw_specs.get_activation_tables

    def patched(arch):
        out = {}
        for name, fns in orig(arch).items():
            fns = set(fns)
            if name != "natural_log_exp_and_others":
                fns.discard(AF.Exp)
                fns.discard(AF.Identity)
                fns.discard(AF.Ln)
            out[name] = fns
        return out

    bacc_mod.get_activation_tables = patched
    bacc_mod._act_tables_patched = True


def _emit_body(nc, tc, t):
    with ExitStack() as ctx:
        singles = ctx.enter_context(tc.tile_pool(name="singles", bufs=1))
        xtp = ctx.enter_context(tc.tile_pool(name="xtp", bufs=8))
        probs = ctx.enter_context(tc.tile_pool(name="probs", bufs=6))
        small = ctx.enter_context(tc.tile_pool(name="small", bufs=2))
        osb = ctx.enter_context(tc.tile_pool(name="osb", bufs=2))
        spP = ctx.enter_context(tc.tile_pool(name="spP", bufs=2, space="PSUM"))
        avP = ctx.enter_context(tc.tile_pool(name="avP", bufs=1, space="PSUM"))
        wpP = ctx.enter_context(tc.tile_pool(name="wpP", bufs=2, space="PSUM"))

        # PE warm-up source tile: memset FIRST so GpSimd produces it before
        # the vS memsets (the warm-up burst must start during the DMA head)
        warm = singles.tile([128, 128], BF16, name="warm")
        nc.gpsimd.memset(warm[:], 0.5)

        # --- constants (DMA emission deferred: the tiny biases go right
        # behind the head-gating wq0/xq00 pair; the 544KB tria/trib go after
        # the whole upfront block since the first tri matmul is at ~40us) ---
        tria_s = singles.tile([128, 128], BF16)
        trib_s = singles.tile([128, 4, QB], BF16)
        bq_s = singles.tile([128, 4], F32)
        bk_s = singles.tile([128, 4], F32)

        def load_biases():
            nc.sync.dma_start(bq_s[:], t["bqd"][:])
            nc.sync.dma_start(bk_s[:], t["bkd"][:])

        def load_tri():
            nc.sync.dma_start(tria_s[:], t["tria"][:])
            for j in range(4):  # split 512KB across queues
                nc.sync.dma_start(trib_s[:, j, :], t["trib"][:, j, :])

        xr = {
            "q": t["xqt"].rearrange("(a p) tok -> p a tok", p=128),
            "k": t["xkt"].rearrange("(a p) tok -> p a tok", p=128),
            "v": t["xvt"].rearrange("(a p) tok -> p a tok", p=128),
        }

        # weights: persistent, per-dc-chunk tiles so the first matmul only
        # waits on one 128KB chunk (emission order controls DMA order)
        w_s = {}

        def get_w(which, dc):
            key = (which, dc)
            if key not in w_s:
                w = singles.tile([128, C], BF16, name=f"w_{which}_{dc}")
                if which == "q" and dc == 0:
                    # the head-gating chunk: halve its arrival latency by
                    # splitting across two queues
                    nc.sync.dma_start(w[:, : C // 2], t["wqt"][:, 0, : C // 2])
                    nc.sync.dma_start(w[:, C // 2 :], t["wqt"][:, 0, C // 2 :])
                else:
                    nc.sync.dma_start(w[:], t["w" + which + "t"][:, dc, :])
                w_s[key] = w
            return w_s[key]

        def get_wo(co):
            key = ("o", co)
            if key not in w_s:
                w = singles.tile([128, D], BF16, name=f"w_o_{co}")
                nc.sync.dma_start(w[:], t["wot"][:, co, :])
                w_s[key] = w
            return w_s[key]

        # --- persistent activations ---
        qT = {}  # (co, tb) -> [128, 512] c-partition, tokens free
        kT = {}
        for co in range(4):
            for tb in range(NTB):
                qT[co, tb] = singles.tile([128, TB], BF16, tag=f"qT_{co}_{tb}", name=f"qT_{co}_{tb}")
                kT[co, tb] = singles.tile([128, TB], BF16, tag=f"kT_{co}_{tb}", name=f"kT_{co}_{tb}")
        # vS: [128 keys, 8 heads, 128]; col 64 = ones (softmax denominator),
        # cols 65-127 zero-padded so the AV lhsT is 128 wide (enables FWL)
        vS = {}
        for kc in range(NKC):
            vS[kc] = singles.tile([128, HL, 128], BF16, tag=f"v_{kc}", name=f"v_{kc}")
            nc.gpsimd.memset(vS[kc][:, :, 64:128], 0.0)
            nc.gpsimd.memset(vS[kc][:, :, 64:65], 1.0)
        aT = {}  # (co, qb) -> [128, 512]
        for co in range(4):
            for qb in range(NTB):
                aT[co, qb] = singles.tile([128, QB], BF16, tag=f"aT_{co}_{qb}", name=f"aT_{co}_{qb}")

        # xt chunk cache: (which, tb, dc) loaded once, used by both co-halves
        xcache = {}

        _xbufs = {"q": 24, "k": 16, "v": 16}

        def get_xt(which, tb, dc):
            key = (which, tb, dc)
            if key not in xcache:
                xt = xtp.tile(
                    [128, TB], BF16, tag=f"x{which}", bufs=_xbufs[which],
                    name=f"x_{which}_{tb}_{dc}",
                )
                if which == "q" and tb == 0 and dc == 0:
                    nc.sync.dma_start(xt[:, : TB // 2], xr["q"][:, 0, : TB // 2])
                    nc.sync.dma_start(xt[:, TB // 2 :], xr["q"][:, 0, TB // 2 : TB])
                else:
                    nc.sync.dma_start(xt[:], xr[which][:, dc, tb * TB : (tb + 1) * TB])
                xcache[key] = xt
            return xcache[key]

        def prefetch_x(which, tb):
            for dc in range(DCH):
                get_xt(which, tb, dc)

        # --- projection / output-projection quanta (1 PSUM bank each,
        # double-buffered through wpP so eviction overlaps the next quantum) ---
        def proj_qk_co(which, tb, co):
            ps = wpP.tile([128, QB], F32, tag="wp", name=f"psP_{which}_{tb}_{co}")
            for dc in range(DCH):
                w = get_w(which, dc)
                xt = get_xt(which, tb, dc)
                nc.tensor.matmul(
                    ps[:],
                    w[:, co * 128 : (co + 1) * 128],
                    xt[:],
                    start=(dc == 0),
                    stop=(dc == DCH - 1),
                )
            b_s = bq_s if which == "q" else bk_s
            dest = qT if which == "q" else kT
            nc.vector.tensor_scalar_add(dest[co, tb][:], ps[:], b_s[:, co : co + 1])

        def proj_v_kl(tb, kl):
            ps = wpP.tile([128, QB], F32, tag="wp", name=f"psV_{tb}_{kl}")
            for dc in range(DCH):
                w = get_w("v", dc)
                xt = get_xt("v", tb, dc)
                nc.tensor.matmul(
                    ps[:],
                    xt[:, kl * 128 : (kl + 1) * 128],
                    w[:],
                    start=(dc == 0),
                    stop=(dc == DCH - 1),
                )
            nc.vector.tensor_copy(
                vS[tb * 4 + kl][:, :, 0:64],
                ps[:].rearrange("p (h e) -> p h e", h=HL),
            )

        def outproj_half(qb, qc, do2):
            ps = wpP.tile([128, QB], F32, tag="wp", name=f"psC_{qb}_{qc}_{do2}")
            for co in range(4):
                nc.tensor.matmul(
                    ps[:],
                    aT[co, qb][:, qc * 128 : (qc + 1) * 128],
                    get_wo(co)[:, do2 * 512 : (do2 + 1) * 512],
                    start=(co == 0),
                    stop=(co == 3),
                )
            ob = osb.tile([128, QB], BF16, tag="ob", name=f"ob_{qb}_{qc}_{do2}")
            nc.vector.tensor_copy(ob[:], ps[:])
            rows = slice(qb * QB + qc * 128, qb * QB + (qc + 1) * 128)
            if qb == 3:
                # tail transfers have no compute to hide under and a single
                # queue drains 128KB at only ~14GB/s (~9us): split across 4
                for k in range(4):
                    cols = slice(do2 * 512 + k * 128, do2 * 512 + (k + 1) * 128)
                    nc.sync.dma_start(
                        t["out_p"][rows, cols], ob[:, k * 128 : (k + 1) * 128]
                    )
            else:
                nc.sync.dma_start(
                    t["out_p"][rows, do2 * 512 : (do2 + 1) * 512], ob[:]
                )

        quanta = deque()

        def pop_quanta(n):
            for _ in range(min(n, len(quanta))):
                quanta.popleft()()

        def pop_one():
            if quanta:
                quanta.popleft()()

        # --- softmax normalization, deferred so its ACT ops slot into the
        # NEXT hp's exp stream instead of clogging the ACT FIFO at the
        # hp boundary. pending_norm = (av, co, qb) ---
        pending_norm = [None]

        def emit_norm():
            if pending_norm[0] is None:
                return
            av, co, nqb = pending_norm[0]
            pending_norm[0] = None
            lnt = small.tile([1, 2, QB], F32, tag="lnt", name=f"lnt_{nqb}_{co}")
            nc.scalar.activation(lnt[:], av[64:65, :, :], AF.Ln)
            rec = small.tile([1, 2, QB], BF16, tag="rec", name=f"rec_{nqb}_{co}")
            nc.scalar.activation(rec[:], lnt[:], AF.Exp, scale=-1.0)
            for hi in range(2):
                po = hi * 64
                bcs = small.tile([128, QB], BF16, tag="bcs", name=f"bcs_{nqb}_{co}_{hi}")
                nc.gpsimd.partition_broadcast(bcs[:, :], rec[0:1, hi, :])
                nc.vector.tensor_mul(
                    aT[co, nqb][po : po + 64, :],
                    av[0:64, hi, :],
                    bcs[po : po + 64, :],
                )

        # --- attention for one query block ---
        def attention(qb, quanta_per_hp):
            n_kc = (qb + 1) * 4
            for hp in range(4):  # heads h0=2hp (par 0-63), h1 (par 64-127)
                co = hp
                av = avP.tile([128, 2, QB], F32, tag="av", name=f"av_{qb}_{hp}")

                def attn_v(kc, pt, off):
                    for hi in range(2):
                        nc.tensor.matmul(
                            av[:, hi, off:],
                            vS[kc][:, 2 * hp + hi, :],
                            pt[:, hi, off:],
                            start=(kc == 0),
                            stop=(kc == n_kc - 1),
                        )

                pend = deque()  # (kc, pt, off) whose exp may still be in flight
                for kc in range(n_kc):
                    j = kc - qb * 4
                    # columns < 128*j of a diagonal chunk are fully masked
                    off = 128 * j if j >= 1 else 0
                    sp = spP.tile([128, 2, QB], F32, tag="spb", name=f"sp_{qb}_{hp}_{kc}")
                    for hi in range(2):
                        po = hi * 64
                        nc.tensor.matmul(
                            sp[:, hi, off:],
                            kT[co, kc // 4][po : po + 64, (kc % 4) * 128 : (kc % 4 + 1) * 128],
                            qT[co, qb][po : po + 64, off:],
                            start=True,
                            stop=(j < 0),
                        )
                    if j >= 0:
                        # only the 128-wide diagonal sub-block needs the causal
                        # ramp; columns beyond it are fully unmasked (their
                        # accumulation group is closed by the narrow tri stop)
                        for hi in range(2):
                            nc.tensor.matmul(
                                sp[:, hi, off : off + 128],
                                tria_s[:],
                                trib_s[:, j, off : off + 128],
                                start=False,
                                stop=True,
                                skip_group_check=True,
                            )
                    pt = probs.tile([128, 2, QB], BF16, tag="pt", name=f"pt_{qb}_{hp}_{kc}")
                    nc.scalar.activation(
                        pt[:, :, off:], sp[:, :, off:], AF.Exp, scale=0.125
                    )
                    if kc == 1:
                        emit_norm()  # previous hp's softmax chain
                    pend.append((kc, pt, off))
                    if len(pend) > 5:
                        attn_v(*pend.popleft())
                    if kc % (4 if qb < 2 else 3) == 2 and kc != n_kc - 1:
                        pop_one()  # sprinkle independent PE work into the loop
                while pend:
                    attn_v(*pend.popleft())
                pending_norm[0] = (av, co, qb)
                pop_quanta(quanta_per_hp)

        # ---------------- schedule ----------------
        # PE warm-up: dummy matmuls spanning the DMA-bound head (~10us) so the
        # HAM clock-gate reaches and STAYS at 2.4GHz until real work arrives
        wps = avP.tile([128, 2, QB], F32, tag="av", name="ps_warm")
        for i in range(110):
            nc.tensor.matmul(
                wps[:, 0, 0:128], warm[:], warm[:], start=(i == 0), stop=(i == 109)
            )
        # upfront: emit all phase-one DMAs first, interleaved in priority
        # order (wq+xq, wk+xk, wv+xv) so the 16 queues drain usefully, then
        # the q/k/v(tb0) quanta find everything cached or in flight
        for which in ("q", "k", "v"):
            for dc in range(DCH):
                get_w(which, dc)
                get_xt(which, 0, dc)
                if which == "q" and dc == 0:
                    load_biases()  # tiny; needed by the first evacuations
        load_tri()  # bulky; not needed until attention(0)
        for co in range(4):
            proj_qk_co("q", 0, co)
        for co in range(4):
            proj_qk_co("k", 0, co)
        for kl in range(4):
            proj_v_kl(0, kl)
        for co in range(4):
            get_wo(co)  # prefetch wo; first used by outproj(0) during qb1

        for qb in range(NTB):
            # enqueue work that becomes available / needed later; x chunks are
            # prefetched at enqueue time so quanta never stall on DMA latency.
            # q(tb) is deferred to phase tb-1 (it is only needed at qb=tb), and
            # outproj halves are pushed late, to fill the ACT-bound late phases
            # with PE work.
            if qb < 3:
                tb = qb + 1
                for which in ("q", "k"):
                    prefetch_x(which, tb)
                    for co in range(4):
                        quanta.append(lambda w=which, tb=tb, co=co: proj_qk_co(w, tb, co))
                prefetch_x("v", tb)
                for kl in range(4):
                    quanta.append(lambda tb=tb, kl=kl: proj_v_kl(tb, kl))
            op_sched = {2: [(0, 0, 8)], 3: [(1, 0, 8), (2, 0, 8)]}
            for oqb, lo, hi in op_sched.get(qb, []):
                for idx in range(lo, hi):
                    qc, do2 = idx // 2, idx % 2
                    quanta.append(
                        lambda oqb=oqb, qc=qc, do2=do2: outproj_half(oqb, qc, do2)
                    )
            attention(qb, quanta_per_hp=(len(quanta) + 3) // 4)
            pop_quanta(len(quanta))  # next phase depends on this phase's quanta
        # drain remaining quanta + final output projection
        emit_norm()  # qb3/hp3's deferred softmax chain
        pop_quanta(len(quanta))
        for qc in range(4):
            for do2 in range(2):
                outproj_half(3, qc, do2)


_PROG = None


def _program():
    global _PROG
    if _PROG is not None:
        return _PROG
    _patch_act_tables()
    nc = bacc.Bacc()
    t = {}
    t["xqt"] = nc.dram_tensor("xqt", [D, S], BF16, kind="ExternalInput")
    t["xkt"] = nc.dram_tensor("xkt", [D, S], BF16, kind="ExternalInput")
    t["xvt"] = nc.dram_tensor("xvt", [D, S], BF16, kind="ExternalInput")
    t["wqt"] = nc.dram_tensor("wqt", [128, DCH, C], BF16, kind="ExternalInput")
    t["wkt"] = nc.dram_tensor("wkt", [128, DCH, C], BF16, kind="ExternalInput")
    t["wvt"] = nc.dram_tensor("wvt", [128, DCH, C], BF16, kind="ExternalInput")
    t["wot"] = nc.dram_tensor("wot", [128, 4, D], BF16, kind="ExternalInput")
    t["bqd"] = nc.dram_tensor("bqd", [128, 4], F32, kind="ExternalInput")
    t["bkd"] = nc.dram_tensor("bkd", [128, 4], F32, kind="ExternalInput")
    t["tria"] = nc.dram_tensor("tria", [128, 128], BF16, kind="ExternalInput")
    t["trib"] = nc.dram_tensor("trib", [128, 4, QB], BF16, kind="ExternalInput")
    t["out_p"] = nc.dram_tensor("out_p", [S, D], BF16, kind="ExternalOutput")
    with tile.TileContext(nc) as tc:
        _emit_body(nc, tc, t)
    nc.compile()
    _PROG = nc
    return nc


def _host_tri():
    import ml_dtypes

    i = np.arange(128)[:, None]
    tria = (16.0 * (i <= np.arange(128)[None, :])).astype(ml_dtypes.bfloat16)
    trib = np.zeros((128, 4, QB), np.float32)
    q = np.arange(QB)[None, :]
    for j in range(4):
        trib[:, j, :] = -15.0 * ((np.arange(128)[:, None] + 128 * j) > q)
    return tria, trib.astype(ml_dtypes.bfloat16)


def prepare_in_maps(Q, K, V, mask, Wq, bq, Wk, bk, Wv, bv, Wo, bo):
    import ml_dtypes

    BF = ml_dtypes.bfloat16
    tria, trib = _host_tri()

    def wslice(W, g):  # [128, 8, 512] lhsT layout of W_slice.T
        Wg = W[g * C : (g + 1) * C, :]  # [512, 1024]
        return np.ascontiguousarray(
            Wg.T.reshape(DCH, 128, C).transpose(1, 0, 2)
        ).astype(BF)

    def woslice(Wo_, g):  # [128, 4, 1024]
        Wg = Wo_[:, g * C : (g + 1) * C]  # [1024, 512]
        return np.ascontiguousarray(
            Wg.T.reshape(4, 128, D).transpose(1, 0, 2)
        ).astype(BF)

    def bslice(b, g):  # [128, 4]
        return np.ascontiguousarray(b[g * C : (g + 1) * C].reshape(4, 128).T).astype(
            np.float32
        )

    in_maps = []
    for core in range(NCORES):
        b, g = core // 2, core % 2
        in_maps.append(
            {
                "xqt": np.ascontiguousarray(np.asarray(Q)[b].T).astype(BF),
                "xkt": np.ascontiguousarray(np.asarray(K)[b].T).astype(BF),
                "xvt": np.ascontiguousarray(np.asarray(V)[b].T).astype(BF),
                "wqt": wslice(np.asarray(Wq), g),
                "wkt": wslice(np.asarray(Wk), g),
                "wvt": wslice(np.asarray(Wv), g),
                "wot": woslice(np.asarray(Wo), g),
                "bqd": bslice(np.asarray(bq), g),
                "bkd": bslice(np.asarray(bk), g),
                "tria": tria,
                "trib": trib,
            }
        )

    return in_maps


def gather_output(results, Wo, bv, bo):
    parts = [np.asarray(r["out_p"], dtype=np.float32) for r in results]
    const = (np.asarray(Wo) @ np.asarray(bv) + np.asarray(bo)).astype(np.float32)
    return np.stack(
        [parts[2 * b] + parts[2 * b + 1] + const for b in range(B)]
    ).astype(np.float32)


def kernel(Q, K, V, mask, Wq, bq, Wk, bk, Wv, bv, Wo, bo):
    nc = _program()
    in_maps = prepare_in_maps(Q, K, V, mask, Wq, bq, Wk, bk, Wv, bv, Wo, bo)
    res = run_bass_kernel_spmd(nc, in_maps, list(range(NCORES)))
    return gather_output(res.results, Wo, bv, bo)


# revision 83
# speedup vs baseline: 1.0380x; 1.0380x over previous
"""Multi-head attention (B=4, S=2048, D=1024, H=16) on 8 TRN2 NeuronCores.

Sharding: core = (batch b = core//2, head-group g = core%2). Each core runs
8 heads (512 channels) of one batch element end-to-end; the host sums the two
head-group partials per batch and adds the constant bias term (bo + Wo@bv).

v2: all matmul operands bf16 (1 cyc/row on PE at any free size, half DMA);
causal offsets exact at 128 granularity; softmax reciprocal via ACT
exp(-ln(den)) (no DVE 8-cyc/elem reciprocal); projections of K/V token-blocks
and the output projection are emitted as "quanta" interleaved into the
attention loop so the PE never idles (and never HAM-rethrottles) while the
ACT engine works through the exp stream.

Device layouts (host-prepped):
  xqt/xkt/xvt [1024, 2048] bf16   input.T per batch
  wqt/wkt/wvt [128, 8, 512] bf16  W_slice.T as [d_par, d_chunk, c]
  wot         [128, 4, 1024] bf16 WoT_slice as [c_par, c_chunk, dout]
  bq/bk       [128, 4] f32        per-partition bias (pre-softmax biases only)
  tria        [128, 128] bf16     16*(i<=k)      -- causal ramp, lhsT
  trib        [128, 4, 512] bf16  -15*(i+128j>q) -- causal ramp, rhs
Output: out_p [2048, 1024] bf16 partial (no bias).
"""

from collections import deque
from contextlib import ExitStack

import numpy as np

import concourse.bacc as bacc
import concourse.bass as bass
import concourse.mybir as mybir
import concourse.tile as tile
from concourse.bass_utils import run_bass_kernel_spmd

B, S, D, H, DK = 4, 2048, 1024, 16, 64
HL, C = 8, 512  # heads / channels per core
NCORES = 8
TB = 512  # token block for projections
QB = 512  # query block for attention
NTB = S // TB  # 4
NKC = S // 128  # 16 key chunks
DCH = D // 128  # 8 d-chunks
F32 = mybir.dt.float32
BF16 = mybir.dt.bfloat16
AF = mybir.ActivationFunctionType


def _patch_act_tables():
    """Confine Exp/Identity/Ln to the natural_log_exp_and_others table set so
    the act-table-load pass emits a single ACT_TABLE_LOAD instead of
    ping-ponging between the exp set and the ln set on every softmax
    normalization (measured 33 loads = 42us of ScalarE time). Only set
    *contents* are edited -- dict order (the act_func_set_id space) is kept."""
    import concourse.bacc as bacc_mod
    import concourse.hw_specs as hw_specs

    if getattr(bacc_mod, "_act_tables_patched", False):
        return
    orig = hw_specs.get_activation_tables

    def patched(arch):
        out = {}
        for name, fns in orig(arch).items():
            fns = set(fns)
            if name != "natural_log_exp_and_others":
                fns.discard(AF.Exp)
                fns.discard(AF.Identity)
                fns.discard(AF.Ln)
            out[name] = fns
        return out

    bacc_mod.get_activation_tables = patched
    bacc_mod._act_tables_patched = True


def _emit_body(nc, tc, t):
    with ExitStack() as ctx:
        singles = ctx.enter_context(tc.tile_pool(name="singles", bufs=1))
        xtp = ctx.enter_context(tc.tile_pool(name="xtp", bufs=8))
        probs = ctx.enter_context(tc.tile_pool(name="probs", bufs=6))
        small = ctx.enter_context(tc.tile_pool(name="small", bufs=2))
        osb = ctx.enter_context(tc.tile_pool(name="osb", bufs=2))
        spP = ctx.enter_context(tc.tile_pool(name="spP", bufs=2, space="PSUM"))
        avP = ctx.enter_context(tc.tile_pool(name="avP", bufs=1, space="PSUM"))
        wpP = ctx.enter_context(tc.tile_pool(name="wpP", bufs=2, space="PSUM"))

        # PE warm-up source tile: memset FIRST so GpSimd produces it before
        # the vS memsets (the warm-up burst must start during the DMA head)
        warm = singles.tile([128, 128], BF16, name="warm")
        nc.gpsimd.memset(warm[:], 0.5)

        # --- constants (DMA emission deferred: the tiny biases go right
        # behind the head-gating wq0/xq00 pair; the 544KB tria/trib go after
        # the whole upfront block since the first tri matmul is at ~40us) ---
        tria_s = singles.tile([128, 128], BF16)
        trib_s = singles.tile([128, 4, QB], BF16)
        bq_s = singles.tile([128, 4], F32)
        bk_s = singles.tile([128, 4], F32)

        def load_biases():
            nc.sync.dma_start(bq_s[:], t["bqd"][:])
            nc.sync.dma_start(bk_s[:], t["bkd"][:])

        def load_tri():
            nc.sync.dma_start(tria_s[:], t["tria"][:])
            for j in range(4):  # split 512KB across queues
                nc.sync.dma_start(trib_s[:, j, :], t["trib"][:, j, :])

        xr = {
            "q": t["xqt"].rearrange("(a p) tok -> p a tok", p=128),
            "k": t["xkt"].rearrange("(a p) tok -> p a tok", p=128),
            "v": t["xvt"].rearrange("(a p) tok -> p a tok", p=128),
        }

        # weights: persistent, per-dc-chunk tiles so the first matmul only
        # waits on one 128KB chunk (emission order controls DMA order)
        w_s = {}

        def get_w(which, dc):
            key = (which, dc)
            if key not in w_s:
                w = singles.tile([128, C], BF16, name=f"w_{which}_{dc}")
                if which == "q" and dc == 0:
                    # the head-gating chunk: halve its arrival latency by
                    # splitting across two queues
                    nc.sync.dma_start(w[:, : C // 2], t["wqt"][:, 0, : C // 2])
                    nc.sync.dma_start(w[:, C // 2 :], t["wqt"][:, 0, C // 2 :])
                else:
                    nc.sync.dma_start(w[:], t["w" + which + "t"][:, dc, :])
                w_s[key] = w
            return w_s[key]

        def get_wo(co):
            key = ("o", co)
            if key not in w_s:
                w = singles.tile([128, D], BF16, name=f"w_o_{co}")
                nc.sync.dma_start(w[:], t["wot"][:, co, :])
                w_s[key] = w
            return w_s[key]

        # --- persistent activations ---
        qT = {}  # (co, tb) -> [128, 512] c-partition, tokens free
        kT = {}
        for co in range(4):
            for tb in range(NTB):
                qT[co, tb] = singles.tile([128, TB], BF16, tag=f"qT_{co}_{tb}", name=f"qT_{co}_{tb}")
                kT[co, tb] = singles.tile([128, TB], BF16, tag=f"kT_{co}_{tb}", name=f"kT_{co}_{tb}")
        # vS: [128 keys, 8 heads, 128]; col 64 = ones (softmax denominator),
        # cols 65-127 zero-padded so the AV lhsT is 128 wide (enables FWL)
        vS = {}
        for kc in range(NKC):
            vS[kc] = singles.tile([128, HL, 128], BF16, tag=f"v_{kc}", name=f"v_{kc}")
            nc.gpsimd.memset(vS[kc][:, :, 64:128], 0.0)
            nc.gpsimd.memset(vS[kc][:, :, 64:65], 1.0)
        aT = {}  # (co, qb) -> [128, 512]
        for co in range(4):
            for qb in range(NTB):
                aT[co, qb] = singles.tile([128, QB], BF16, tag=f"aT_{co}_{qb}", name=f"aT_{co}_{qb}")

        # xt chunk cache: (which, tb, dc) loaded once, used by both co-halves
        xcache = {}

        _xbufs = {"q": 24, "k": 16, "v": 16}

        def get_xt(which, tb, dc):
            key = (which, tb, dc)
            if key not in xcache:
                xt = xtp.tile(
                    [128, TB], BF16, tag=f"x{which}", bufs=_xbufs[which],
                    name=f"x_{which}_{tb}_{dc}",
                )
                if which == "q" and tb == 0 and dc == 0:
                    nc.sync.dma_start(xt[:, : TB // 2], xr["q"][:, 0, : TB // 2])
                    nc.sync.dma_start(xt[:, TB // 2 :], xr["q"][:, 0, TB // 2 : TB])
                else:
                    nc.sync.dma_start(xt[:], xr[which][:, dc, tb * TB : (tb + 1) * TB])
                xcache[key] = xt
            return xcache[key]

        def prefetch_x(which, tb):
            for dc in range(DCH):
                get_xt(which, tb, dc)

        # --- projection / output-projection quanta (1 PSUM bank each,
        # double-buffered through wpP so eviction overlaps the next quantum) ---
        def proj_qk_co(which, tb, co):
            ps = wpP.tile([128, QB], F32, tag="wp", name=f"psP_{which}_{tb}_{co}")
            for dc in range(DCH):
                w = get_w(which, dc)
                xt = get_xt(which, tb, dc)
                nc.tensor.matmul(
                    ps[:],
                    w[:, co * 128 : (co + 1) * 128],
                    xt[:],
                    start=(dc == 0),
                    stop=(dc == DCH - 1),
                )
            b_s = bq_s if which == "q" else bk_s
            dest = qT if which == "q" else kT
            nc.vector.tensor_scalar_add(dest[co, tb][:], ps[:], b_s[:, co : co + 1])

        def proj_v_kl(tb, kl):
            ps = wpP.tile([128, QB], F32, tag="wp", name=f"psV_{tb}_{kl}")
            for dc in range(DCH):
                w = get_w("v", dc)
                xt = get_xt("v", tb, dc)
                nc.tensor.matmul(
                    ps[:],
                    xt[:, kl * 128 : (kl + 1) * 128],
                    w[:],
                    start=(dc == 0),
                    stop=(dc == DCH - 1),
                )
            nc.vector.tensor_copy(
                vS[tb * 4 + kl][:, :, 0:64],
                ps[:].rearrange("p (h e) -> p h e", h=HL),
            )

        def outproj_half(qb, qc, do2):
            ps = wpP.tile([128, QB], F32, tag="wp", name=f"psC_{qb}_{qc}_{do2}")
            for co in range(4):
                nc.tensor.matmul(
                    ps[:],
                    aT[co, qb][:, qc * 128 : (qc + 1) * 128],
                    get_wo(co)[:, do2 * 512 : (do2 + 1) * 512],
                    start=(co == 0),
                    stop=(co == 3),
                )
            ob = osb.tile([128, QB], BF16, tag="ob", name=f"ob_{qb}_{qc}_{do2}")
            nc.vector.tensor_copy(ob[:], ps[:])
            nc.sync.dma_start(
                t["out_p"][
                    qb * QB + qc * 128 : qb * QB + (qc + 1) * 128,
                    do2 * 512 : (do2 + 1) * 512,
                ],
                ob[:],
            )

        quanta = deque()

        def pop_quanta(n):
            for _ in range(min(n, len(quanta))):
                quanta.popleft()()

        def pop_one():
            if quanta:
                quanta.popleft()()

        # --- softmax normalization, deferred so its ACT ops slot into the
        # NEXT hp's exp stream instead of clogging the ACT FIFO at the
        # hp boundary. pending_norm = (av, co, qb) ---
        pending_norm = [None]

        def emit_norm():
            if pending_norm[0] is None:
                return
            av, co, nqb = pending_norm[0]
            pending_norm[0] = None
            lnt = small.tile([1, 2, QB], F32, tag="lnt", name=f"lnt_{nqb}_{co}")
            nc.scalar.activation(lnt[:], av[64:65, :, :], AF.Ln)
            rec = small.tile([1, 2, QB], BF16, tag="rec", name=f"rec_{nqb}_{co}")
            nc.scalar.activation(rec[:], lnt[:], AF.Exp, scale=-1.0)
            for hi in range(2):
                po = hi * 64
                bcs = small.tile([128, QB], BF16, tag="bcs", name=f"bcs_{nqb}_{co}_{hi}")
                nc.gpsimd.partition_broadcast(bcs[:, :], rec[0:1, hi, :])
                nc.vector.tensor_mul(
                    aT[co, nqb][po : po + 64, :],
                    av[0:64, hi, :],
                    bcs[po : po + 64, :],
                )

        # --- attention for one query block ---
        def attention(qb, quanta_per_hp):
            n_kc = (qb + 1) * 4
            for hp in range(4):  # heads h0=2hp (par 0-63), h1 (par 64-127)
                co = hp
                av = avP.tile([128, 2, QB], F32, tag="av", name=f"av_{qb}_{hp}")

                def attn_v(kc, pt, off):
                    for hi in range(2):
                        nc.tensor.matmul(
                            av[:, hi, off:],
                            vS[kc][:, 2 * hp + hi, :],
                            pt[:, hi, off:],
                            start=(kc == 0),
                            stop=(kc == n_kc - 1),
                        )

                pend = deque()  # (kc, pt, off) whose exp may still be in flight
                for kc in range(n_kc):
                    j = kc - qb * 4
                    # columns < 128*j of a diagonal chunk are fully masked
                    off = 128 * j if j >= 1 else 0
                    sp = spP.tile([128, 2, QB], F32, tag="spb", name=f"sp_{qb}_{hp}_{kc}")
                    for hi in range(2):
                        po = hi * 64
                        nc.tensor.matmul(
                            sp[:, hi, off:],
                            kT[co, kc // 4][po : po + 64, (kc % 4) * 128 : (kc % 4 + 1) * 128],
                            qT[co, qb][po : po + 64, off:],
                            start=True,
                            stop=(j < 0),
                        )
                    if j >= 0:
                        # only the 128-wide diagonal sub-block needs the causal
                        # ramp; columns beyond it are fully unmasked (their
                        # accumulation group is closed by the narrow tri stop)
                        for hi in range(2):
                            nc.tensor.matmul(
                                sp[:, hi, off : off + 128],
                                tria_s[:],
                                trib_s[:, j, off : off + 128],
                                start=False,
                                stop=True,
                                skip_group_check=True,
                            )
                    pt = probs.tile([128, 2, QB], BF16, tag="pt", name=f"pt_{qb}_{hp}_{kc}")
                    nc.scalar.activation(
                        pt[:, :, off:], sp[:, :, off:], AF.Exp, scale=0.125
                    )
                    if kc == 1:
                        emit_norm()  # previous hp's softmax chain
                    pend.append((kc, pt, off))
                    if len(pend) > 5:
                        attn_v(*pend.popleft())
                    if kc % (4 if qb < 2 else 3) == 2 and kc != n_kc - 1:
                        pop_one()  # sprinkle independent PE work into the loop
                while pend:
                    attn_v(*pend.popleft())
                pending_norm[0] = (av, co, qb)
                pop_quanta(quanta_per_hp)

        # ---------------- schedule ----------------
        # PE warm-up: dummy matmuls spanning the DMA-bound head (~10us) so the
        # HAM clock-gate reaches and STAYS at 2.4GHz until real work arrives
        wps = avP.tile([128, 2, QB], F32, tag="av", name="ps_warm")
        for i in range(110):
            nc.tensor.matmul(
                wps[:, 0, 0:128], warm[:], warm[:], start=(i == 0), stop=(i == 109)
            )
        # upfront: emit all phase-one DMAs first, interleaved in priority
        # order (wq+xq, wk+xk, wv+xv) so the 16 queues drain usefully, then
        # the q/k/v(tb0) quanta find everything cached or in flight
        for which in ("q", "k", "v"):
            for dc in range(DCH):
                get_w(which, dc)
                get_xt(which, 0, dc)
                if which == "q" and dc == 0:
                    load_biases()  # tiny; needed by the first evacuations
        load_tri()  # bulky; not needed until attention(0)
        for co in range(4):
            proj_qk_co("q", 0, co)
        for co in range(4):
            proj_qk_co("k", 0, co)
        for kl in range(4):
            proj_v_kl(0, kl)
        for co in range(4):
            get_wo(co)  # prefetch wo; first used by outproj(0) during qb1

        for qb in range(NTB):
            # enqueue work that becomes available / needed later; x chunks are
            # prefetched at enqueue time so quanta never stall on DMA latency.
            # q(tb) is deferred to phase tb-1 (it is only needed at qb=tb), and
            # outproj halves are pushed late, to fill the ACT-bound late phases
            # with PE work.
            if qb < 3:
                tb = qb + 1
                for which in ("q", "k"):
                    prefetch_x(which, tb)
                    for co in range(4):
                        quanta.append(lambda w=which, tb=tb, co=co: proj_qk_co(w, tb, co))
                prefetch_x("v", tb)
                for kl in range(4):
                    quanta.append(lambda tb=tb, kl=kl: proj_v_kl(tb, kl))
            op_sched = {2: [(0, 0, 8)], 3: [(1, 0, 8), (2, 0, 8)]}
            for oqb, lo, hi in op_sched.get(qb, []):
                for idx in range(lo, hi):
                    qc, do2 = idx // 2, idx % 2
                    quanta.append(
                        lambda oqb=oqb, qc=qc, do2=do2: outproj_half(oqb, qc, do2)
                    )
            attention(qb, quanta_per_hp=(len(quanta) + 3) // 4)
            pop_quanta(len(quanta))  # next phase depends on this phase's quanta
        # drain remaining quanta + final output projection
        emit_norm()  # qb3/hp3's deferred softmax chain
        pop_quanta(len(quanta))
        for qc in range(4):
            for do2 in range(2):
                outproj_half(3, qc, do2)


_PROG = None


def _program():
    global _PROG
    if _PROG is not None:
        return _PROG
    _patch_act_tables()
    nc = bacc.Bacc()
    t = {}
    t["xqt"] = nc.dram_tensor("xqt", [D, S], BF16, kind="ExternalInput")
    t["xkt"] = nc.dram_tensor("xkt", [D, S], BF16, kind="ExternalInput")
    t["xvt"] = nc.dram_tensor("xvt", [D, S], BF16, kind="ExternalInput")
    t["wqt"] = nc.dram_tensor("wqt", [128, DCH, C], BF16, kind="ExternalInput")
    t["wkt"] = nc.dram_tensor("wkt", [128, DCH, C], BF16, kind="ExternalInput")
    t["wvt"] = nc.dram_tensor("wvt", [128, DCH, C], BF16, kind="ExternalInput")
    t["wot"] = nc.dram_tensor("wot", [128, 4, D], BF16, kind="ExternalInput")
    t["bqd"] = nc.dram_tensor("bqd", [128, 4], F32, kind="ExternalInput")
    t["bkd"] = nc.dram_tensor("bkd", [128, 4], F32, kind="ExternalInput")
    t["tria"] = nc.dram_tensor("tria", [128, 128], BF16, kind="ExternalInput")
    t["trib"] = nc.dram_tensor("trib", [128, 4, QB], BF16, kind="ExternalInput")
    t["out_p"] = nc.dram_tensor("out_p", [S, D], BF16, kind="ExternalOutput")
    with tile.TileContext(nc) as tc:
        _emit_body(nc, tc, t)
    nc.compile()
    _PROG = nc
    return nc


def _host_tri():
    import ml_dtypes

    i = np.arange(128)[:, None]
    tria = (16.0 * (i <= np.arange(128)[None, :])).astype(ml_dtypes.bfloat16)
    trib = np.zeros((128, 4, QB), np.float32)
    q = np.arange(QB)[None, :]
    for j in range(4):
        trib[:, j, :] = -15.0 * ((np.arange(128)[:, None] + 128 * j) > q)
    return tria, trib.astype(ml_dtypes.bfloat16)


def prepare_in_maps(Q, K, V, mask, Wq, bq, Wk, bk, Wv, bv, Wo, bo):
    import ml_dtypes

    BF = ml_dtypes.bfloat16
    tria, trib = _host_tri()

    def wslice(W, g):  # [128, 8, 512] lhsT layout of W_slice.T
        Wg = W[g * C : (g + 1) * C, :]  # [512, 1024]
        return np.ascontiguousarray(
            Wg.T.reshape(DCH, 128, C).transpose(1, 0, 2)
        ).astype(BF)

    def woslice(Wo_, g):  # [128, 4, 1024]
        Wg = Wo_[:, g * C : (g + 1) * C]  # [1024, 512]
        return np.ascontiguousarray(
            Wg.T.reshape(4, 128, D).transpose(1, 0, 2)
        ).astype(BF)

    def bslice(b, g):  # [128, 4]
        return np.ascontiguousarray(b[g * C : (g + 1) * C].reshape(4, 128).T).astype(
            np.float32
        )

    in_maps = []
    for core in range(NCORES):
        b, g = core // 2, core % 2
        in_maps.append(
            {
                "xqt": np.ascontiguousarray(np.asarray(Q)[b].T).astype(BF),
                "xkt": np.ascontiguousarray(np.asarray(K)[b].T).astype(BF),
                "xvt": np.ascontiguousarray(np.asarray(V)[b].T).astype(BF),
                "wqt": wslice(np.asarray(Wq), g),
                "wkt": wslice(np.asarray(Wk), g),
                "wvt": wslice(np.asarray(Wv), g),
                "wot": woslice(np.asarray(Wo), g),
                "bqd": bslice(np.asarray(bq), g),
                "bkd": bslice(np.asarray(bk), g),
                "tria": tria,
                "trib": trib,
            }
        )

    return in_maps


def gather_output(results, Wo, bv, bo):
    parts = [np.asarray(r["out_p"], dtype=np.float32) for r in results]
    const = (np.asarray(Wo) @ np.asarray(bv) + np.asarray(bo)).astype(np.float32)
    return np.stack(
        [parts[2 * b] + parts[2 * b + 1] + const for b in range(B)]
    ).astype(np.float32)


def kernel(Q, K, V, mask, Wq, bq, Wk, bk, Wv, bv, Wo, bo):
    nc = _program()
    in_maps = prepare_in_maps(Q, K, V, mask, Wq, bq, Wk, bk, Wv, bv, Wo, bo)
    res = run_bass_kernel_spmd(nc, in_maps, list(range(NCORES)))
    return gather_output(res.results, Wo, bv, bo)


# revision 84
# speedup vs baseline: 1.0479x; 1.0096x over previous
"""Multi-head attention (B=4, S=2048, D=1024, H=16) on 8 TRN2 NeuronCores.

Sharding: core = (batch b = core//2, head-group g = core%2). Each core runs
8 heads (512 channels) of one batch element end-to-end; the host sums the two
head-group partials per batch and adds the constant bias term (bo + Wo@bv).

v2: all matmul operands bf16 (1 cyc/row on PE at any free size, half DMA);
causal offsets exact at 128 granularity; softmax reciprocal via ACT
exp(-ln(den)) (no DVE 8-cyc/elem reciprocal); projections of K/V token-blocks
and the output projection are emitted as "quanta" interleaved into the
attention loop so the PE never idles (and never HAM-rethrottles) while the
ACT engine works through the exp stream.

Device layouts (host-prepped):
  xqt/xkt/xvt [1024, 2048] bf16   input.T per batch
  wqt/wkt/wvt [128, 8, 512] bf16  W_slice.T as [d_par, d_chunk, c]
  wot         [128, 4, 1024] bf16 WoT_slice as [c_par, c_chunk, dout]
  bq/bk       [128, 4] f32        per-partition bias (pre-softmax biases only)
  tria        [128, 128] bf16     16*(i<=k)      -- causal ramp, lhsT
  trib        [128, 4, 512] bf16  -15*(i+128j>q) -- causal ramp, rhs
Output: out_p [2048, 1024] bf16 partial (no bias).
"""

from collections import deque
from contextlib import ExitStack

import numpy as np

import concourse.bacc as bacc
import concourse.bass as bass
import concourse.mybir as mybir
import concourse.tile as tile
from concourse.bass_utils import run_bass_kernel_spmd

B, S, D, H, DK = 4, 2048, 1024, 16, 64
HL, C = 8, 512  # heads / channels per core
NCORES = 8
TB = 512  # token block for projections
QB = 512  # query block for attention
NTB = S // TB  # 4
NKC = S // 128  # 16 key chunks
DCH = D // 128  # 8 d-chunks
F32 = mybir.dt.float32
BF16 = mybir.dt.bfloat16
AF = mybir.ActivationFunctionType


def _patch_act_tables():
    """Confine Exp/Identity/Ln to the natural_log_exp_and_others table set so
    the act-table-load pass emits a single ACT_TABLE_LOAD instead of
    ping-ponging between the exp set and the ln set on every softmax
    normalization (measured 33 loads = 42us of ScalarE time). Only set
    *contents* are edited -- dict order (the act_func_set_id space) is kept."""
    import concourse.bacc as bacc_mod
    import concourse.hw_specs as hw_specs

    if getattr(bacc_mod, "_act_tables_patched", False):
        return
    orig = hw_specs.get_activation_tables

    def patched(arch):
        out = {}
        for name, fns in orig(arch).items():
            fns = set(fns)
            if name != "natural_log_exp_and_others":
                fns.discard(AF.Exp)
                fns.discard(AF.Identity)
                fns.discard(AF.Ln)
            out[name] = fns
        return out

    bacc_mod.get_activation_tables = patched
    bacc_mod._act_tables_patched = True


def _emit_body(nc, tc, t):
    with ExitStack() as ctx:
        singles = ctx.enter_context(tc.tile_pool(name="singles", bufs=1))
        xtp = ctx.enter_context(tc.tile_pool(name="xtp", bufs=8))
        probs = ctx.enter_context(tc.tile_pool(name="probs", bufs=6))
        small = ctx.enter_context(tc.tile_pool(name="small", bufs=2))
        # bufs=4: each ob's 128KB store drains ~6us on its single DMA queue;
        # a 2-deep ring lets bursts of outproj quanta stall their PSUM
        # evacuation (and thus the wpP ring / PE) behind an in-flight store
        osb = ctx.enter_context(tc.tile_pool(name="osb", bufs=4))
        spP = ctx.enter_context(tc.tile_pool(name="spP", bufs=2, space="PSUM"))
        avP = ctx.enter_context(tc.tile_pool(name="avP", bufs=1, space="PSUM"))
        wpP = ctx.enter_context(tc.tile_pool(name="wpP", bufs=2, space="PSUM"))

        # PE warm-up source tile: memset FIRST so GpSimd produces it before
        # the vS memsets (the warm-up burst must start during the DMA head)
        warm = singles.tile([128, 128], BF16, name="warm")
        nc.gpsimd.memset(warm[:], 0.5)

        # --- constants (DMA emission deferred: the tiny biases go right
        # behind the head-gating wq0/xq00 pair; the 544KB tria/trib go after
        # the whole upfront block since the first tri matmul is at ~40us) ---
        tria_s = singles.tile([128, 128], BF16)
        trib_s = singles.tile([128, 4, QB], BF16)
        bq_s = singles.tile([128, 4], F32)
        bk_s = singles.tile([128, 4], F32)

        def load_biases():
            nc.sync.dma_start(bq_s[:], t["bqd"][:])
            nc.sync.dma_start(bk_s[:], t["bkd"][:])

        def load_tri():
            nc.sync.dma_start(tria_s[:], t["tria"][:])
            for j in range(4):  # split 512KB across queues
                nc.sync.dma_start(trib_s[:, j, :], t["trib"][:, j, :])

        xr = {
            "q": t["xqt"].rearrange("(a p) tok -> p a tok", p=128),
            "k": t["xkt"].rearrange("(a p) tok -> p a tok", p=128),
            "v": t["xvt"].rearrange("(a p) tok -> p a tok", p=128),
        }

        # weights: persistent, per-dc-chunk tiles so the first matmul only
        # waits on one 128KB chunk (emission order controls DMA order)
        w_s = {}

        def get_w(which, dc):
            key = (which, dc)
            if key not in w_s:
                w = singles.tile([128, C], BF16, name=f"w_{which}_{dc}")
                if which == "q" and dc == 0:
                    # the head-gating chunk: halve its arrival latency by
                    # splitting across two queues
                    nc.sync.dma_start(w[:, : C // 2], t["wqt"][:, 0, : C // 2])
                    nc.sync.dma_start(w[:, C // 2 :], t["wqt"][:, 0, C // 2 :])
                else:
                    nc.sync.dma_start(w[:], t["w" + which + "t"][:, dc, :])
                w_s[key] = w
            return w_s[key]

        def get_wo(co):
            key = ("o", co)
            if key not in w_s:
                w = singles.tile([128, D], BF16, name=f"w_o_{co}")
                nc.sync.dma_start(w[:], t["wot"][:, co, :])
                w_s[key] = w
            return w_s[key]

        # --- persistent activations ---
        qT = {}  # (co, tb) -> [128, 512] c-partition, tokens free
        kT = {}
        for co in range(4):
            for tb in range(NTB):
                qT[co, tb] = singles.tile([128, TB], BF16, tag=f"qT_{co}_{tb}", name=f"qT_{co}_{tb}")
                kT[co, tb] = singles.tile([128, TB], BF16, tag=f"kT_{co}_{tb}", name=f"kT_{co}_{tb}")
        # vS: [128 keys, 8 heads, 128]; col 64 = ones (softmax denominator),
        # cols 65-127 zero-padded so the AV lhsT is 128 wide (enables FWL)
        vS = {}
        for kc in range(NKC):
            vS[kc] = singles.tile([128, HL, 128], BF16, tag=f"v_{kc}", name=f"v_{kc}")
            nc.gpsimd.memset(vS[kc][:, :, 64:128], 0.0)
            nc.gpsimd.memset(vS[kc][:, :, 64:65], 1.0)
        aT = {}  # (co, qb) -> [128, 512]
        for co in range(4):
            for qb in range(NTB):
                aT[co, qb] = singles.tile([128, QB], BF16, tag=f"aT_{co}_{qb}", name=f"aT_{co}_{qb}")

        # xt chunk cache: (which, tb, dc) loaded once, used by both co-halves
        xcache = {}

        _xbufs = {"q": 24, "k": 16, "v": 16}

        def get_xt(which, tb, dc):
            key = (which, tb, dc)
            if key not in xcache:
                xt = xtp.tile(
                    [128, TB], BF16, tag=f"x{which}", bufs=_xbufs[which],
                    name=f"x_{which}_{tb}_{dc}",
                )
                if which == "q" and tb == 0 and dc == 0:
                    nc.sync.dma_start(xt[:, : TB // 2], xr["q"][:, 0, : TB // 2])
                    nc.sync.dma_start(xt[:, TB // 2 :], xr["q"][:, 0, TB // 2 : TB])
                else:
                    nc.sync.dma_start(xt[:], xr[which][:, dc, tb * TB : (tb + 1) * TB])
                xcache[key] = xt
            return xcache[key]

        def prefetch_x(which, tb):
            for dc in range(DCH):
                get_xt(which, tb, dc)

        # --- projection / output-projection quanta (1 PSUM bank each,
        # double-buffered through wpP so eviction overlaps the next quantum) ---
        def proj_qk_co(which, tb, co):
            ps = wpP.tile([128, QB], F32, tag="wp", name=f"psP_{which}_{tb}_{co}")
            for dc in range(DCH):
                w = get_w(which, dc)
                xt = get_xt(which, tb, dc)
                nc.tensor.matmul(
                    ps[:],
                    w[:, co * 128 : (co + 1) * 128],
                    xt[:],
                    start=(dc == 0),
                    stop=(dc == DCH - 1),
                )
            b_s = bq_s if which == "q" else bk_s
            dest = qT if which == "q" else kT
            nc.vector.tensor_scalar_add(dest[co, tb][:], ps[:], b_s[:, co : co + 1])

        def proj_v_kl(tb, kl):
            ps = wpP.tile([128, QB], F32, tag="wp", name=f"psV_{tb}_{kl}")
            for dc in range(DCH):
                w = get_w("v", dc)
                xt = get_xt("v", tb, dc)
                nc.tensor.matmul(
                    ps[:],
                    xt[:, kl * 128 : (kl + 1) * 128],
                    w[:],
                    start=(dc == 0),
                    stop=(dc == DCH - 1),
                )
            nc.vector.tensor_copy(
                vS[tb * 4 + kl][:, :, 0:64],
                ps[:].rearrange("p (h e) -> p h e", h=HL),
            )

        def outproj_half(qb, qc, do2):
            ps = wpP.tile([128, QB], F32, tag="wp", name=f"psC_{qb}_{qc}_{do2}")
            for co in range(4):
                nc.tensor.matmul(
                    ps[:],
                    aT[co, qb][:, qc * 128 : (qc + 1) * 128],
                    get_wo(co)[:, do2 * 512 : (do2 + 1) * 512],
                    start=(co == 0),
                    stop=(co == 3),
                )
            ob = osb.tile([128, QB], BF16, tag="ob", name=f"ob_{qb}_{qc}_{do2}")
            nc.vector.tensor_copy(ob[:], ps[:])
            nc.sync.dma_start(
                t["out_p"][
                    qb * QB + qc * 128 : qb * QB + (qc + 1) * 128,
                    do2 * 512 : (do2 + 1) * 512,
                ],
                ob[:],
            )

        quanta = deque()

        def pop_quanta(n):
            for _ in range(min(n, len(quanta))):
                quanta.popleft()()

        def pop_one():
            if quanta:
                quanta.popleft()()

        # --- softmax normalization, deferred so its ACT ops slot into the
        # NEXT hp's exp stream instead of clogging the ACT FIFO at the
        # hp boundary. pending_norm = (av, co, qb) ---
        pending_norm = [None]

        def emit_norm():
            if pending_norm[0] is None:
                return
            av, co, nqb = pending_norm[0]
            pending_norm[0] = None
            lnt = small.tile([1, 2, QB], F32, tag="lnt", name=f"lnt_{nqb}_{co}")
            nc.scalar.activation(lnt[:], av[64:65, :, :], AF.Ln)
            rec = small.tile([1, 2, QB], BF16, tag="rec", name=f"rec_{nqb}_{co}")
            nc.scalar.activation(rec[:], lnt[:], AF.Exp, scale=-1.0)
            for hi in range(2):
                po = hi * 64
                bcs = small.tile([128, QB], BF16, tag="bcs", name=f"bcs_{nqb}_{co}_{hi}")
                nc.gpsimd.partition_broadcast(bcs[:, :], rec[0:1, hi, :])
                nc.vector.tensor_mul(
                    aT[co, nqb][po : po + 64, :],
                    av[0:64, hi, :],
                    bcs[po : po + 64, :],
                )

        # --- attention for one query block ---
        def attention(qb, quanta_per_hp):
            n_kc = (qb + 1) * 4
            for hp in range(4):  # heads h0=2hp (par 0-63), h1 (par 64-127)
                co = hp
                av = avP.tile([128, 2, QB], F32, tag="av", name=f"av_{qb}_{hp}")

                def attn_v(kc, pt, off):
                    for hi in range(2):
                        nc.tensor.matmul(
                            av[:, hi, off:],
                            vS[kc][:, 2 * hp + hi, :],
                            pt[:, hi, off:],
                            start=(kc == 0),
                            stop=(kc == n_kc - 1),
                        )

                pend = deque()  # (kc, pt, off) whose exp may still be in flight
                for kc in range(n_kc):
                    j = kc - qb * 4
                    # columns < 128*j of a diagonal chunk are fully masked
                    off = 128 * j if j >= 1 else 0
                    sp = spP.tile([128, 2, QB], F32, tag="spb", name=f"sp_{qb}_{hp}_{kc}")
                    for hi in range(2):
                        po = hi * 64
                        nc.tensor.matmul(
                            sp[:, hi, off:],
                            kT[co, kc // 4][po : po + 64, (kc % 4) * 128 : (kc % 4 + 1) * 128],
                            qT[co, qb][po : po + 64, off:],
                            start=True,
                            stop=(j < 0),
                        )
                    if j >= 0:
                        # only the 128-wide diagonal sub-block needs the causal
                        # ramp; columns beyond it are fully unmasked (their
                        # accumulation group is closed by the narrow tri stop)
                        for hi in range(2):
                            nc.tensor.matmul(
                                sp[:, hi, off : off + 128],
                                tria_s[:],
                                trib_s[:, j, off : off + 128],
                                start=False,
                                stop=True,
                                skip_group_check=True,
                            )
                    pt = probs.tile([128, 2, QB], BF16, tag="pt", name=f"pt_{qb}_{hp}_{kc}")
                    nc.scalar.activation(
                        pt[:, :, off:], sp[:, :, off:], AF.Exp, scale=0.125
                    )
                    if kc == 1:
                        emit_norm()  # previous hp's softmax chain
                    pend.append((kc, pt, off))
                    if len(pend) > 5:
                        attn_v(*pend.popleft())
                    if kc % (4 if qb < 2 else 3) == 2 and kc != n_kc - 1:
                        pop_one()  # sprinkle independent PE work into the loop
                while pend:
                    attn_v(*pend.popleft())
                pending_norm[0] = (av, co, qb)
                pop_quanta(quanta_per_hp)

        # ---------------- schedule ----------------
        # PE warm-up: dummy matmuls spanning the DMA-bound head (~10us) so the
        # HAM clock-gate reaches and STAYS at 2.4GHz until real work arrives
        wps = avP.tile([128, 2, QB], F32, tag="av", name="ps_warm")
        for i in range(110):
            nc.tensor.matmul(
                wps[:, 0, 0:128], warm[:], warm[:], start=(i == 0), stop=(i == 109)
            )
        # upfront: emit all phase-one DMAs first, interleaved in priority
        # order (wq+xq, wk+xk, wv+xv) so the 16 queues drain usefully, then
        # the q/k/v(tb0) quanta find everything cached or in flight
        for which in ("q", "k", "v"):
            for dc in range(DCH):
                get_w(which, dc)
                get_xt(which, 0, dc)
                if which == "q" and dc == 0:
                    load_biases()  # tiny; needed by the first evacuations
        load_tri()  # bulky; not needed until attention(0)
        for co in range(4):
            proj_qk_co("q", 0, co)
        for co in range(4):
            proj_qk_co("k", 0, co)
        for kl in range(4):
            proj_v_kl(0, kl)
        for co in range(4):
            get_wo(co)  # prefetch wo; first used by outproj(0) during qb1

        for qb in range(NTB):
            # enqueue work that becomes available / needed later; x chunks are
            # prefetched at enqueue time so quanta never stall on DMA latency.
            # q(tb) is deferred to phase tb-1 (it is only needed at qb=tb), and
            # outproj halves are pushed late, to fill the ACT-bound late phases
            # with PE work.
            if qb < 3:
                tb = qb + 1
                for which in ("q", "k"):
                    prefetch_x(which, tb)
                    for co in range(4):
                        quanta.append(lambda w=which, tb=tb, co=co: proj_qk_co(w, tb, co))
                prefetch_x("v", tb)
                for kl in range(4):
                    quanta.append(lambda tb=tb, kl=kl: proj_v_kl(tb, kl))
            op_sched = {2: [(0, 0, 8)], 3: [(1, 0, 8), (2, 0, 8)]}
            for oqb, lo, hi in op_sched.get(qb, []):
                for idx in range(lo, hi):
                    qc, do2 = idx // 2, idx % 2
                    quanta.append(
                        lambda oqb=oqb, qc=qc, do2=do2: outproj_half(oqb, qc, do2)
                    )
            attention(qb, quanta_per_hp=(len(quanta) + 3) // 4)
            pop_quanta(len(quanta))  # next phase depends on this phase's quanta
        # drain remaining quanta + final output projection
        emit_norm()  # qb3/hp3's deferred softmax chain
        pop_quanta(len(quanta))
        for qc in range(4):
            for do2 in range(2):
                outproj_half(3, qc, do2)


_PROG = None


def _program():
    global _PROG
    if _PROG is not None:
        return _PROG
    _patch_act_tables()
    nc = bacc.Bacc()
    t = {}
    t["xqt"] = nc.dram_tensor("xqt", [D, S], BF16, kind="ExternalInput")
    t["xkt"] = nc.dram_tensor("xkt", [D, S], BF16, kind="ExternalInput")
    t["xvt"] = nc.dram_tensor("xvt", [D, S], BF16, kind="ExternalInput")
    t["wqt"] = nc.dram_tensor("wqt", [128, DCH, C], BF16, kind="ExternalInput")
    t["wkt"] = nc.dram_tensor("wkt", [128, DCH, C], BF16, kind="ExternalInput")
    t["wvt"] = nc.dram_tensor("wvt", [128, DCH, C], BF16, kind="ExternalInput")
    t["wot"] = nc.dram_tensor("wot", [128, 4, D], BF16, kind="ExternalInput")
    t["bqd"] = nc.dram_tensor("bqd", [128, 4], F32, kind="ExternalInput")
    t["bkd"] = nc.dram_tensor("bkd", [128, 4], F32, kind="ExternalInput")
    t["tria"] = nc.dram_tensor("tria", [128, 128], BF16, kind="ExternalInput")
    t["trib"] = nc.dram_tensor("trib", [128, 4, QB], BF16, kind="ExternalInput")
    t["out_p"] = nc.dram_tensor("out_p", [S, D], BF16, kind="ExternalOutput")
    with tile.TileContext(nc) as tc:
        _emit_body(nc, tc, t)
    nc.compile()
    _PROG = nc
    return nc


def _host_tri():
    import ml_dtypes

    i = np.arange(128)[:, None]
    tria = (16.0 * (i <= np.arange(128)[None, :])).astype(ml_dtypes.bfloat16)
    trib = np.zeros((128, 4, QB), np.float32)
    q = np.arange(QB)[None, :]
    for j in range(4):
        trib[:, j, :] = -15.0 * ((np.arange(128)[:, None] + 128 * j) > q)
    return tria, trib.astype(ml_dtypes.bfloat16)


def prepare_in_maps(Q, K, V, mask, Wq, bq, Wk, bk, Wv, bv, Wo, bo):
    import ml_dtypes

    BF = ml_dtypes.bfloat16
    tria, trib = _host_tri()

    def wslice(W, g):  # [128, 8, 512] lhsT layout of W_slice.T
        Wg = W[g * C : (g + 1) * C, :]  # [512, 1024]
        return np.ascontiguousarray(
            Wg.T.reshape(DCH, 128, C).transpose(1, 0, 2)
        ).astype(BF)

    def woslice(Wo_, g):  # [128, 4, 1024]
        Wg = Wo_[:, g * C : (g + 1) * C]  # [1024, 512]
        return np.ascontiguousarray(
            Wg.T.reshape(4, 128, D).transpose(1, 0, 2)
        ).astype(BF)

    def bslice(b, g):  # [128, 4]
        return np.ascontiguousarray(b[g * C : (g + 1) * C].reshape(4, 128).T).astype(
            np.float32
        )

    in_maps = []
    for core in range(NCORES):
        b, g = core // 2, core % 2
        in_maps.append(
            {
                "xqt": np.ascontiguousarray(np.asarray(Q)[b].T).astype(BF),
                "xkt": np.ascontiguousarray(np.asarray(K)[b].T).astype(BF),
                "xvt": np.ascontiguousarray(np.asarray(V)[b].T).astype(BF),
                "wqt": wslice(np.asarray(Wq), g),
                "wkt": wslice(np.asarray(Wk), g),
                "wvt": wslice(np.asarray(Wv), g),
                "wot": woslice(np.asarray(Wo), g),
                "bqd": bslice(np.asarray(bq), g),
                "bkd": bslice(np.asarray(bk), g),
                "tria": tria,
                "trib": trib,
            }
        )

    return in_maps


def gather_output(results, Wo, bv, bo):
    parts = [np.asarray(r["out_p"], dtype=np.float32) for r in results]
    const = (np.asarray(Wo) @ np.asarray(bv) + np.asarray(bo)).astype(np.float32)
    return np.stack(
        [parts[2 * b] + parts[2 * b + 1] + const for b in range(B)]
    ).astype(np.float32)


def kernel(Q, K, V, mask, Wq, bq, Wk, bk, Wv, bv, Wo, bo):
    nc = _program()
    in_maps = prepare_in_maps(Q, K, V, mask, Wq, bq, Wk, bk, Wv, bv, Wo, bo)
    res = run_bass_kernel_spmd(nc, in_maps, list(range(NCORES)))
    return gather_output(res.results, Wo, bv, bo)
